# revision 10
# baseline (speedup 1.0000x reference)
"""Trainium2 Bass kernel for nn_Decoder_14697378087362.

Attention-LSTM decoder (LAS-style): T=400 sequential steps, each running
3 LSTM cells (hidden 512), masked attention over S=2048 encoder positions
(dim 128), and a small tied-embedding output projection.

Strategy: data-parallel over batch (B=64 -> 8 samples/core on 8 cores).
All weights + keys/values SBUF-resident in bf16, f32 PSUM accumulation.
LSTM matmuls run batch-stationary (weights streamed as rhs at full clock).
Attention energies for all 8 samples are accumulated into one [8,S] PSUM
tile via one-hot stationary columns; softmax uses exp (no max subtraction;
energies are O(1)), the denominator comes from a ones-column appended to
values, and attention context is accumulated the same one-hot way.
emb[tok] is teacher-forced, so it is gathered/transposed on the host;
generated = argmax(logits + gumbel noise) is computed on the host too.
"""

import sys
import numpy as np

for _p in ("/opt/trn_rl_repo", "/root/.axon_site/_ro/trn_rl_repo"):
    if _p not in sys.path:
        sys.path.insert(0, _p)

import ml_dtypes

BF16 = ml_dtypes.bfloat16

# problem shapes (hardcoded per contract)
NINP, NC, NH = 34, 128, 512
B, S, T = 64, 2048, 400
NCORES = 8
BS = B // NCORES          # samples per core
ST = S // 128             # s-tiles per sample
G4 = 4 * NH               # 2048 gate width

_PROG_CACHE = {}


# ---------------------------------------------------------------------------
# Post-pass: this container's walrus encodes at most one sem wait per
# instruction ('Too many sync wait commands' otherwise). Move excess waits
# onto same-engine NoOps inserted immediately before the instruction.
# ---------------------------------------------------------------------------
def _split_sync_waits(nc, limit=1):
    import concourse.mybir as mybir

    n_split = 0
    for f in nc.m.functions:
        for bb in f.blocks:
            il = bb.instructions
            i = 0
            while i < len(il):
                inst = il[i]
                si = getattr(inst, "sync_info", None)
                if si is not None and len(si.on_wait) > limit:
                    waits = list(si.on_wait)
                    head, tail = waits[:-limit], waits[-limit:]
                    nops = []
                    for k in range(0, len(head), limit):
                        nop = mybir.InstNoOp(name=f"{inst.name}-ws{k}", ins=[], outs=[])
                        nop.engine = inst.engine
                        nop.sync_info = mybir.SyncInfo(on_wait=head[k:k + limit], on_update=[])
                        nops.append(nop)
                    inst.sync_info = mybir.SyncInfo(on_wait=tail, on_update=list(si.on_update))
                    for j, nop in enumerate(nops):
                        il.insert(i + j, nop)
                    i += len(nops)
                    n_split += 1
                i += 1
    return n_split


# ---------------------------------------------------------------------------
# device program
# ---------------------------------------------------------------------------
def _build_program(n_steps, with_gate_bias, with_bq, with_bc):
    import concourse.bass as bass
    import concourse.tile as tile
    import concourse.mybir as mybir
    from contextlib import ExitStack

    f32 = mybir.dt.float32
    bf16 = mybir.dt.bfloat16
    AF = mybir.ActivationFunctionType

    nc = bass.Bass("TRN2", target_bir_lowering=False, debug=False, num_devices=NCORES)

    # ---- DRAM I/O ----------------------------------------------------------
    d_in = {}

    def din(name, shape, dt):
        d_in[name] = nc.dram_tensor(name, list(shape), dt, kind="ExternalInput")
        return d_in[name]

    # lstm weight streams, stored pre-transposed [K, 2048] as [Kt, 128, 2048]
    w0e = din("w0e", (1, 128, G4), bf16)        # W0ih[:, :128].T
    w0c = din("w0c", (1, 128, G4), bf16)        # W0ih[:, 128:].T
    w0h = din("w0h", (4, 128, G4), bf16)        # W0hh.T
    w1i = din("w1i", (4, 128, G4), bf16)
    w1h = din("w1h", (4, 128, G4), bf16)
    w2i = din("w2i", (4, 128, G4), bf16)
    w2h = din("w2h", (4, 128, G4), bf16)
    wqT = din("wqT", (4, 128, NC), bf16)        # wq.T [512,128]
    wchT = din("wchT", (4, 128, NC), bf16)      # wc[:, :512].T
    wccT = din("wccT", (1, 128, NC), bf16)      # wc[:, 512:].T
    embT = din("embT", (NC, NINP), f32)         # emb.T (f32 for the logit mm)
    keysT_d = din("keysT", (BS, NC, S), bf16)   # per-sample keys.T (C on parts)
    vals_d = din("valsA", (BS, ST, 128, NC + 1), bf16)  # values + ones col
    mb_d = din("maskb", (BS, S), bf16)          # -1e6 * (1 - mask)
    emb_seq_d = din("embseq", (n_steps, NC, BS), bf16)  # emb[tok].T per step
    hT0_d = din("hT0", (3, NC, 4 * BS), bf16)   # initial h (transposed k-tiles)
    c0_d = din("c0", (3, BS, NH), f32)
    ctxT0_d = din("ctxT0", (NC, BS), bf16)
    ident_d = din("ident", (BS, BS), f32)       # I8 f32 (transpose identity)
    ident_bf_d = din("identbf", (BS, BS), bf16) # I8 bf16 (maskbias matmul)
    if with_gate_bias:
        gbias_d = din("gbias", (1, 3, G4), f32)  # b_ih + b_hh per layer
    if with_bq:
        bq_d = din("bq", (1, NC), f32)
    if with_bc:
        bc_d = din("bc", (1, NC), f32)

    att_d = nc.dram_tensor("att_out", [n_steps, BS, S], f32, kind="ExternalOutput")
    log_d = nc.dram_tensor("log_out", [n_steps, BS, NINP], f32, kind="ExternalOutput")

    with tile.TileContext(nc) as tc, ExitStack() as ctx:
        consts = ctx.enter_context(tc.tile_pool(name="consts", bufs=1))
        sb = ctx.enter_context(tc.tile_pool(name="sb", bufs=2))
        sb_att = ctx.enter_context(tc.tile_pool(name="sb_att", bufs=2))
        ps_big = ctx.enter_context(tc.tile_pool(name="ps_big", bufs=1, space="PSUM"))
        ps1 = ctx.enter_context(tc.tile_pool(name="ps1", bufs=4, space="PSUM"))

        def ps_tile(shape):
            return ps1.tile(list(shape), mybir.dt.float32, tag="ps1", name="pst")

        def load_const(dram, shape, dt):
            t = consts.tile(list(shape), dt, tag=f"c_{dram.name}", name=f"c_{dram.name}")
            nc.sync.dma_start(t[:], dram[:])
            return t

        # ---- resident SBUF tensors ----------------------------------------
        # weight streams: one tile per k-tile [128, 2048]
        def load_w(dram, kt, ncols):
            tl = []
            for k in range(kt):
                t = consts.tile([128, ncols], bf16, tag=f"w_{dram.name}_{k}", name=f"w_{dram.name}_{k}")
                nc.sync.dma_start(t[:], dram[k])
                tl.append(t)
            return tl

        W0E = load_w(w0e, 1, G4)[0]
        W0C = load_w(w0c, 1, G4)[0]
        W0H = load_w(w0h, 4, G4)
        W1I = load_w(w1i, 4, G4)
        W1H = load_w(w1h, 4, G4)
        W2I = load_w(w2i, 4, G4)
        W2H = load_w(w2h, 4, G4)
        WQ = load_w(wqT, 4, NC)
        WCH = load_w(wchT, 4, NC)
        WCC = load_w(wccT, 1, NC)[0]
        EMBT = load_const(embT, (NC, NINP), f32)
        IDENT = load_const(ident_d, (BS, BS), f32)
        IDBF = load_const(ident_bf_d, (BS, BS), bf16)
        MASKB = load_const(mb_d, (BS, S), bf16)

        KEYS = []   # per sample [128, S] bf16
        for b in range(BS):
            t = consts.tile([NC, S], bf16, tag=f"keys{b}", name=f"keys{b}")
            nc.sync.dma_start(t[:], keysT_d[b])
            KEYS.append(t)
        VALS = []   # per sample [ST, 128, 129] -> tile [128, ST*(NC+1)]
        for b in range(BS):
            t = consts.tile([128, ST, NC + 1], bf16, tag=f"vals{b}", name=f"vals{b}")
            nc.sync.dma_start(t[:], vals_d[b].rearrange("st p c -> p st c"))
            VALS.append(t)

        if with_gate_bias:
            GB = load_const(gbias_d, (1, 3, G4), f32)
        if with_bq:
            BQ = load_const(bq_d, (1, NC), f32)
        if with_bc:
            BC = load_const(bc_d, (1, NC), f32)
        ONES1 = consts.tile([1, BS], f32)
        nc.vector.memset(ONES1[:], 1.0)

        # persistent state
        C_ST = []
        HT_ST = []
        for l in range(3):
            c = consts.tile([BS, NH], f32, tag=f"c{l}", name=f"c{l}")
            nc.sync.dma_start(c[:], c0_d[l])
            C_ST.append(c)
            h = consts.tile([NC, 4 * BS], bf16, tag=f"hT{l}", name=f"hT{l}")
            nc.sync.dma_start(h[:], hT0_d[l])
            HT_ST.append(h)
        CTXT = consts.tile([NC, BS], bf16, tag="ctxT")
        nc.sync.dma_start(CTXT[:], ctxT0_d[:])

        # one-hot layouts, zeroed once; only their diagonal columns are
        # rewritten each step
        EOH = consts.tile([128, ST * 8 * BS], bf16, tag="eoh")  # col 64*st+9*b
        nc.vector.memset(EOH[:], 0.0)
        QOH = consts.tile([128, BS * BS], bf16, tag="qoh")      # col 9*b
        nc.vector.memset(QOH[:], 0.0)

        W_I = [None, W1I, W2I]
        W_H = [W0H, W1H, W2H]

        # ---- one decoder step ---------------------------------------------
        def step(t):
            # stage emb[tok_t].T
            xe = sb.tile([NC, BS], bf16, tag="xemb")
            nc.sync.dma_start(xe[:], emb_seq_d[t])

            for l in range(3):
                # gates: 4 one-bank psum tiles [8, 512]
                gp = [ps_tile([BS, NH]) for _ in range(4)]
                for g in range(4):
                    cols = slice(g * NH, (g + 1) * NH)
                    first = True
                    if l == 0:
                        nc.tensor.matmul(gp[g][:], xe[:], W0E[:, cols], start=True, stop=False)
                        nc.tensor.matmul(gp[g][:], CTXT[:], W0C[:, cols], start=False, stop=False)
                        first = False
                    else:
                        for k in range(4):
                            nc.tensor.matmul(gp[g][:], HT_ST[l - 1][:, k * BS:(k + 1) * BS],
                                             W_I[l][k][:, cols], start=first, stop=False)
                            first = False
                    for k in range(4):
                        last = (k == 3) and not with_gate_bias
                        nc.tensor.matmul(gp[g][:], HT_ST[l][:, k * BS:(k + 1) * BS],
                                         W_H[l][k][:, cols], start=False, stop=last)
                    if with_gate_bias:
                        nc.tensor.matmul(gp[g][:], ONES1[:], GB[:, l, g * NH:(g + 1) * NH],
                                         start=False, stop=True)

                i_s = sb.tile([BS, NH], f32, tag="i_s")
                f_s = sb.tile([BS, NH], f32, tag="f_s")
                g_t = sb.tile([BS, NH], f32, tag="g_t")
                o_s = sb.tile([BS, NH], f32, tag="o_s")
                nc.scalar.activation(i_s[:], gp[0][:], AF.Sigmoid)
                nc.scalar.activation(f_s[:], gp[1][:], AF.Sigmoid)
                nc.scalar.activation(g_t[:], gp[2][:], AF.Tanh)
                nc.scalar.activation(o_s[:], gp[3][:], AF.Sigmoid)

                # c = f*c + i*g ; h = o * tanh(c)   (in-place temps)
                nc.vector.tensor_mul(f_s[:], f_s[:], C_ST[l][:])
                nc.vector.tensor_mul(i_s[:], i_s[:], g_t[:])
                nc.vector.tensor_add(C_ST[l][:], f_s[:], i_s[:])
                nc.scalar.activation(g_t[:], C_ST[l][:], AF.Tanh)
                nc.vector.tensor_mul(o_s[:], o_s[:], g_t[:])
                h_l = o_s

                # transpose h -> hT state (4 blocks of [8,128] -> [128,8])
                pht = ps_tile([128, 4 * BS])
                for k in range(4):
                    nc.tensor.transpose(pht[:, k * BS:(k + 1) * BS],
                                        h_l[:, k * 128:(k + 1) * 128], IDENT[:])
                nc.vector.tensor_copy(HT_ST[l][:], pht[:])

            # ---- q = h3 @ wq.T (+bq), one-hot Q ---------------------------
            pq = ps_tile([BS, NC])
            for k in range(4):
                nc.tensor.matmul(pq[:], HT_ST[2][:, k * BS:(k + 1) * BS], WQ[k][:],
                                 start=(k == 0), stop=(k == 3) and not with_bq)
            if with_bq:
                nc.tensor.matmul(pq[:], ONES1[:], BQ[:], start=False, stop=True)
            q_sb = sb.tile([BS, NC], f32, tag="q_sb")
            nc.vector.tensor_copy(q_sb[:], pq[:])
            pqT = ps_tile([NC, BS])
            nc.tensor.transpose(pqT[:], q_sb[:], IDENT[:])
            for b in range(BS):
                nc.vector.tensor_copy(QOH[:, 9 * b:9 * b + 1], pqT[:, b:b + 1])

            # ---- energies: [8, S] psum, one-hot accumulation --------------
            pe = ps_big.tile([BS, S], f32, tag="pE")
            for h in range(4):
                cols = slice(h * 512, (h + 1) * 512)
                nc.tensor.matmul(pe[:, cols], IDBF[:], MASKB[:, cols],
                                 start=True, stop=False)
            for b in range(BS):
                for h in range(4):
                    cols = slice(h * 512, (h + 1) * 512)
                    nc.tensor.matmul(pe[:, cols], QOH[:, 8 * b:8 * b + 8],
                                     KEYS[b][:, cols], start=False,
                                     stop=(b == BS - 1) and (h == 3))
            e_f = sb_att.tile([BS, S], f32, tag="e_f")
            nc.scalar.activation(e_f[:], pe[:], AF.Exp)

            # transpose e -> [128, ST*8], then scatter diag cols into EOH
            pet = ps_tile([128, ST * 8])
            for st in range(ST):
                nc.tensor.transpose(pet[:, st * 8:st * 8 + 8],
                                    e_f[:, st * 128:(st + 1) * 128], IDENT[:])
            pet3 = pet[:].rearrange("p (st b) -> p st b", b=8)
            eoh3 = EOH[:].rearrange("p (st c) -> p st c", c=8 * BS)
            for b in range(BS):
                nc.vector.tensor_copy(eoh3[:, :, 9 * b], pet3[:, :, b])

            # ---- ctx (+ sum via ones column) ------------------------------
            pc = ps_tile([BS, NC + 1])
            n_mm = 0
            for b in range(BS):
                for st in range(ST):
                    n_mm += 1
                    nc.tensor.matmul(pc[:], EOH[:, st * 64 + 8 * b: st * 64 + 8 * b + 8],
                                     VALS[b][:, st], start=(n_mm == 1),
                                     stop=(n_mm == BS * ST))
            recip = sb.tile([BS, 1], f32, tag="recip")
            nc.vector.reciprocal(recip[:], pc[:, NC:NC + 1])
            ctx_sb = sb.tile([BS, NC], f32, tag="ctx_sb")
            nc.vector.tensor_scalar_mul(ctx_sb[:], pc[:, 0:NC], recip[:])
            pcT = ps_tile([NC, BS])
            nc.tensor.transpose(pcT[:], ctx_sb[:], IDENT[:])
            nc.vector.tensor_copy(CTXT[:], pcT[:])

            # attention output = e * (1/sum), scaled in place then DMA'd out
            nc.vector.tensor_scalar_mul(e_f[:], e_f[:], recip[:])
            nc.sync.dma_start(att_d[t], e_f[:])

            # ---- z, logits -------------------------------------------------
            pz = ps_tile([BS, NC])
            for k in range(4):
                nc.tensor.matmul(pz[:], HT_ST[2][:, k * BS:(k + 1) * BS], WCH[k][:],
                                 start=(k == 0), stop=False)
            nc.tensor.matmul(pz[:], CTXT[:], WCC[:], start=False,
                             stop=not with_bc)
            if with_bc:
                nc.tensor.matmul(pz[:], ONES1[:], BC[:], start=False, stop=True)
            z_sb = sb.tile([BS, NC], f32, tag="z_sb")
            nc.scalar.activation(z_sb[:], pz[:], AF.Lrelu, alpha=0.01)
            pzT = ps_tile([NC, BS])
            nc.tensor.transpose(pzT[:], z_sb[:], IDENT[:])
            zT = sb.tile([NC, BS], f32, tag="zT")
            nc.vector.tensor_copy(zT[:], pzT[:])
            pl = ps_tile([BS, NINP])
            nc.tensor.matmul(pl[:], zT[:], EMBT[:], start=True, stop=True)
            log_sb = sb.tile([BS, NINP], f32, tag="log_sb")
            nc.vector.tensor_copy(log_sb[:], pl[:])
            nc.sync.dma_start(log_d[t], log_sb[:])

        for t in range(n_steps):
            step(t)

    _split_sync_waits(nc)
    return nc


# ---------------------------------------------------------------------------
# host side
# ---------------------------------------------------------------------------
def _prep_core_inputs(core, keys, values, lens, emb_seq_T, weights):
    """Build the in_map for one core (samples core*BS .. core*BS+BS)."""
    sl = slice(core * BS, (core + 1) * BS)
    k = keys[sl]                      # [BS, S, NC] f32
    v = values[sl]                    # [BS, S, NC] f32
    ln = lens[sl]                     # [BS]

    keysT = np.ascontiguousarray(k.transpose(0, 2, 1)).astype(BF16)  # [BS,NC,S]
    vaug = np.concatenate(
        [v, np.ones((BS, S, 1), np.float32)], axis=2)                # [BS,S,NC+1]
    vaug = vaug.reshape(BS, ST, 128, NC + 1).astype(BF16)
    maskb = np.where(np.arange(S)[None, :] < ln[:, None], 0.0, -1e6).astype(np.float32).astype(BF16)

    # initial ctx from uniform attention over valid positions
    att0 = (np.arange(S)[None, :] < ln[:, None]).astype(np.float64)
    att0 /= att0.sum(axis=1, keepdims=True)
    ctx0 = np.einsum("bs,bsc->bc", att0, v.astype(np.float64)).astype(np.float32)
    ctxT0 = np.ascontiguousarray(ctx0.T).astype(BF16)                # [NC, BS]

    m = {
        "keysT": keysT,
        "valsA": vaug,
        "maskb": maskb,
        "embseq": np.ascontiguousarray(emb_seq_T[:, :, sl]),         # [T,NC,BS]
        "ctxT0": ctxT0,
    }
    m.update(weights)
    return m


def kernel(**inputs):
    import concourse.bass as bass  # noqa: F401  (ensures repo importable early)
    from concourse.bass_utils import run_bass_kernel_spmd
    import jax

    f32 = np.float32
    keys = np.asarray(inputs["keys"], f32)
    values = np.asarray(inputs["values"], f32)
    lens = np.asarray(inputs["lens"]).astype(np.int64)
    toks = np.asarray(inputs["inputs"]).astype(np.int64)      # [B, T]
    emb = np.asarray(inputs["emb"], f32)
    wq = np.asarray(inputs["wq"], f32)
    bq = np.asarray(inputs["bq"], f32)
    wc = np.asarray(inputs["wc"], f32)
    bc = np.asarray(inputs["bc"], f32)
    b_out = np.asarray(inputs["b_out"], f32)
    n_steps = toks.shape[1]

    wih = [np.asarray(inputs[f"w{l}_ih"], f32) for l in range(3)]
    whh = [np.asarray(inputs[f"w{l}_hh"], f32) for l in range(3)]
    bih = [np.asarray(inputs[f"b{l}_ih"], f32) for l in range(3)]
    bhh = [np.asarray(inputs[f"b{l}_hh"], f32) for l in range(3)]
    h0 = [np.asarray(inputs[f"h0_{l}"], f32) for l in range(3)]
    c0 = [np.asarray(inputs[f"c0_{l}"], f32) for l in range(3)]

    gbias = np.stack([bih[l] + bhh[l] for l in range(3)])[None]  # [1,3,2048]
    with_gate_bias = bool(np.any(gbias))
    with_bq = bool(np.any(bq))
    with_bc = bool(np.any(bc))

    # ---- replicated weight arrays -----------------------------------------
    def kt(a, k):  # [Kt*128, N] -> [Kt, 128, N]
        return np.ascontiguousarray(a).reshape(k, 128, -1)

    weights = {
        "w0e": kt(wih[0][:, :128].T.copy(), 1).astype(BF16),
        "w0c": kt(wih[0][:, 128:].T.copy(), 1).astype(BF16),
        "w0h": kt(whh[0].T.copy(), 4).astype(BF16),
        "w1i": kt(wih[1].T.copy(), 4).astype(BF16),
        "w1h": kt(whh[1].T.copy(), 4).astype(BF16),
        "w2i": kt(wih[2].T.copy(), 4).astype(BF16),
        "w2h": kt(whh[2].T.copy(), 4).astype(BF16),
        "wqT": kt(wq.T.copy(), 4).astype(BF16),
        "wchT": kt(wc[:, :NH].T.copy(), 4).astype(BF16),
        "wccT": kt(wc[:, NH:].T.copy(), 1).astype(BF16),
        "embT": np.ascontiguousarray(emb.T).astype(f32),
        "ident": np.eye(BS, dtype=f32),
        "identbf": np.eye(BS).astype(BF16),
        "c0": np.stack([np.broadcast_to(c0[l], (BS, NH)) for l in range(3)]).astype(f32),
    }
    # initial hT: h0 broadcast [BS,NH] -> transposed k-tiles [NC, 4*BS]
    hT0 = np.zeros((3, NC, 4 * BS), f32)
    for l in range(3):
        hb = np.broadcast_to(h0[l], (BS, NH))
        for k in range(4):
            hT0[l, :, k * BS:(k + 1) * BS] = hb[:, k * 128:(k + 1) * 128].T
    weights["hT0"] = hT0.astype(BF16)
    if with_gate_bias:
        weights["gbias"] = gbias.astype(f32)
    if with_bq:
        weights["bq"] = bq[None, :].astype(f32)
    if with_bc:
        weights["bc"] = bc[None, :].astype(f32)

    # emb[tok].T for every step: [T, NC, B]
    emb_seq_T = np.ascontiguousarray(
        emb[toks].transpose(1, 2, 0)).astype(BF16)

    key = ("prog", n_steps, with_gate_bias, with_bq, with_bc)
    if key not in _PROG_CACHE:
        _PROG_CACHE[key] = _build_program(n_steps, with_gate_bias, with_bq, with_bc)
    nc = _PROG_CACHE[key]

    in_maps = [
        _prep_core_inputs(c, keys, values, lens, emb_seq_T, weights)
        for c in range(NCORES)
    ]
    res = run_bass_kernel_spmd(nc, in_maps, core_ids=list(range(NCORES)))

    logits = np.concatenate([res.results[c]["log_out"] for c in range(NCORES)], axis=1)
    attentions = np.concatenate([res.results[c]["att_out"] for c in range(NCORES)], axis=1)
    logits = logits + b_out[None, None, :]

    # gumbel noise identical to the reference's (threefry is deterministic)
    EPS = 1e-10
    with jax.default_device(jax.devices("cpu")[0]):
        U = np.asarray(jax.random.uniform(jax.random.key(1), (n_steps, B, NINP),
                                          dtype=np.float32))
    gnoise = -np.log(EPS - np.log(U + EPS))
    generated = np.argmax(logits + gnoise, axis=2).astype(np.int32)

    return logits.astype(np.float32), attentions.astype(np.float32), generated
